# revision 55
# baseline (speedup 1.0000x reference)
"""Trainium2 Bass kernel for nn_AttentionLayer (sparse sliding-window attention).

Reference computation (T=1024, B=8, D=512, W=32):
  windows[t, j]  = inputs[t + j]                      (t in [0, T-W), j in [0, W))
  u              = tanh(windows @ weight_W)
  scores[t,j,b]  = u[t,j,b,:] @ proj
  a              = softmax over b  (legacy nn.Softmax dim=1 on [w,B])
  attended[t]    = sum_j a[t,j] * windows[t,j]
  out            = concat([inputs[:W], attended])

Key algebraic collapse: u[t, j] depends only on i = t + j, so per-timestep
  s[i, b]  = tanh(inputs[i, b, :] @ W) @ proj          # [T, B]
  p[i, b]  = exp(s[i,b]) / sum_b' exp(s[i,b'])          # softmax over B
  out[i]   = sum_{k=i-W}^{i-1} p[k] * inputs[k]  for i >= W
This removes the 32x matmul redundancy.  The window sum is a banded-matrix
matmul on TensorE.

Sharding: sequence-parallel over T with a W-row halo; no collectives.
Each of the 8 cores computes 124 attended rows from 155 input rows
(padded to 160).  Softmax denominators are built in column form with
accumulating rank-1 matmuls; 1/den is folded into the band matrix; the
p-scaled band is built via rank-1 outer products of exp(s) against the
band mask.

Compute dtype: bf16 matmuls (f32 PSUM accumulate) — PE runs f32 matmul at
1/4 rate, bf16 at full rate; tolerance is 2e-2 so bf16 is comfortable.
Output is stored bf16 and widened to f32 on the host.
"""

import numpy as np

T, B, D, W = 1024, 8, 512, 32
NCORES = 8
TOUT = (T - W) // NCORES          # 124 attended rows per core
TLOC = 160                        # padded input rows per core (>= TOUT + W - 1 = 155)
NI = TLOC * B                     # 1280 flattened (t, b) columns
DCH = D // 128                    # 4 chunks of the contraction dim
# N-tile split of the 1280-wide moving dim (one PSUM bank = 512 f32)
N_TILES = [(0, 512), (512, 512), (1024, 256)]
# t_in chunks (partition dim of the banded matmul)
T_CHUNKS = [(0, 128), (128, TLOC - 128)]
# the halo chunk (t 128:160) only feeds output rows [96:124) (32-aligned)
C1_T0 = 96
C1_W = TOUT - C1_T0

_CACHE = {}


def _build():
    import concourse.bass as bass
    import concourse.mybir as mybir
    import concourse.tile as tile
    from concourse import bacc

    f32 = mybir.dt.float32
    bf16 = mybir.dt.bfloat16
    AF = mybir.ActivationFunctionType
    ALU = mybir.AluOpType

    nc = bacc.Bacc("TRN2", target_bir_lowering=False, debug=False)

    x_T = nc.dram_tensor("x_T", [D, NI], bf16, kind="ExternalInput")
    x_nat = nc.dram_tensor("x_nat", [TLOC, B * D], bf16, kind="ExternalInput")
    w_in = nc.dram_tensor("w", [D, D], bf16, kind="ExternalInput")
    proj_in = nc.dram_tensor("proj", [D, 1], bf16, kind="ExternalInput")
    at_in = nc.dram_tensor("at_band", [TLOC, TOUT], f32, kind="ExternalInput")
    out_ext = nc.dram_tensor("out", [TOUT, B * D], bf16, kind="ExternalOutput")

    with tile.TileContext(nc) as tc:
        with (
            tc.tile_pool(name="const", bufs=1) as const_pool,
            tc.tile_pool(name="data", bufs=1) as data_pool,
            tc.tile_pool(name="work", bufs=8) as work_pool,
            tc.tile_pool(name="mbp", bufs=16) as mb_pool,
            tc.tile_pool(name="ps", bufs=2, space="PSUM") as psum_pool,
            tc.tile_pool(name="ps_o", bufs=1, space="PSUM") as psum_o_pool,
            tc.tile_pool(name="ps_s", bufs=1, space="PSUM") as psum_s_pool,
            tc.tile_pool(name="ps_att", bufs=4, space="PSUM") as psum_att_pool,
        ):
            ones_bf = const_pool.tile([1, TOUT], bf16)
            nc.vector.memset(ones_bf[:], 1.0)
            # PE warmup: keep TensorE busy during the input-DMA window so the
            # HAM clock gate is at 8/8 when the real matmuls start.
            wm_sink = data_pool.tile([1, 1], f32)
            ps_wm = psum_o_pool.tile([128, TOUT], f32, tag="ps_o", name="ps_wm")
            for _wm in range(8):
                nc.tensor.matmul(ps_wm[0:1, :], lhsT=ones_bf[:, 0:1], rhs=ones_bf[:],
                                 start=True, stop=True)
            nc.any.tensor_copy(wm_sink[:], ps_wm[0:1, 0:1])

            # ---- input loads: W halves first (first LDW needs cols 0:256),
            # then xt N-tile pieces (phase-1 moving operand), then phase-2
            # data; scalar + sync HWDGE queues used for prep parallelism ----
            w_sb = const_pool.tile([128, DCH, D], bf16)         # W[d, e] d-chunked
            w_src = w_in.ap().rearrange("(c p) e -> p c e", p=128)
            nc.scalar.dma_start(w_sb[:, :, 0:256], w_src[:, :, 0:256])
            nc.scalar.dma_start(w_sb[:, :, 256:512], w_src[:, :, 256:512])
            proj_sb = const_pool.tile([128, DCH, 1], bf16)
            nc.scalar.dma_start(proj_sb[:], proj_in.ap().rearrange("(c p) o -> p c o", p=128))
            at_sb0 = const_pool.tile([128, TOUT], f32)
            nc.scalar.dma_start(at_sb0[:], at_in.ap()[0:128, :])
            at_sb1 = const_pool.tile([TLOC - 128, TOUT], f32)
            nc.scalar.dma_start(at_sb1[:], at_in.ap()[128:TLOC, :])
            xt_sb = data_pool.tile([128, DCH, NI], bf16)        # x.T d-chunked
            xt_src = x_T.ap().rearrange("(c p) i -> p c i", p=128)
            for (n0, nsz) in N_TILES:
                nc.sync.dma_start(xt_sb[:, :, n0:n0 + nsz], xt_src[:, :, n0:n0 + nsz])
            xn_sb0 = data_pool.tile([128, B * D], bf16)         # x natural rows 0..127
            nc.sync.dma_start(xn_sb0[:], x_nat.ap()[0:128, :])
            xn_sb1 = data_pool.tile([TLOC - 128, B * D], bf16)  # rows 128..159
            nc.sync.dma_start(xn_sb1[:], x_nat.ap()[128:TLOC, :])

            tanh_sb = data_pool.tile([128, DCH, NI], bf16)      # tanh(u).T
            e_row = data_pool.tile([1, NI], bf16)               # exp(s) row
            r_col0 = data_pool.tile([128, 1], f32)
            r_col1 = data_pool.tile([TLOC - 128, 1], f32)
            ats0 = data_pool.tile([128, TOUT], f32)             # band * 1/den
            ats1 = data_pool.tile([TLOC - 128, TOUT], f32)

            e_btv = e_row.rearrange("p (t b) -> p b t", b=B)    # strided view
            r_cols = [r_col0, r_col1]
            mbs = {}

            def softmax_chunk(ci):
                """Outer products + denominator for one t-chunk.  Each b's
                den rank-1 matmul is emitted right after its outer product
                with the SAME stationary operand, so the LDWEIGHTS is shared;
                then reciprocal, band scaling, and wide broadcast mask TTs."""
                c0, csz = T_CHUNKS[ci]
                ps_d_full = psum_s_pool.tile([128, 512], f32, tag="ps_s",
                                             name=f"ps_d{ci}")
                ps_d = ps_d_full[:, 0:1]
                if ci == 0:
                    # pack 4 b's outer products per psum bank (disjoint column
                    # ranges, one accumulation group -> safe under both
                    # per-element and bank-wide has_written semantics), then
                    # mask all 4 with ONE wide TT (band operand broadcast).
                    ps_os = []
                    for g in range(2):
                        ps_o = psum_o_pool.tile([128, 4 * TOUT], f32, tag="ps_o",
                                                name=f"ps_o0_{g}")
                        for bi in range(4):
                            b = 4 * g + bi
                            nc.tensor.matmul(
                                ps_o[:csz, bi * TOUT:(bi + 1) * TOUT],
                                lhsT=e_btv[:, b, c0:c0 + csz],
                                rhs=ones_bf[:],
                                start=(bi == 0), stop=(bi == 3),
                                skip_group_check=True,
                            )
                            nc.tensor.matmul(
                                ps_d[:csz, :],
                                lhsT=e_btv[:, b, c0:c0 + csz],
                                rhs=ones_bf[:, 0:1],
                                start=(b == 0), stop=(b == B - 1),
                                skip_group_check=True,
                            )
                        ps_os.append(ps_o)
                    nc.vector.reciprocal(r_cols[ci][:csz, :], ps_d[:csz, :])
                    nc.vector.tensor_scalar_mul(ats0[:], at_sb0[:], r_col0[:])
                    for g in range(2):
                        mb_g = mb_pool.tile([128, 4, TOUT], bf16, tag="mb0",
                                            name=f"mb0_{g}")
                        nc.vector.tensor_tensor(
                            mb_g[:csz, :, :],
                            ps_os[g][:csz, :].rearrange("p (b c) -> p b c", b=4),
                            ats0[:csz, None, :].to_broadcast((csz, 4, TOUT)),
                            ALU.mult,
                        )
                        for bi in range(4):
                            mbs[(4 * g + bi, 0)] = mb_g[:, bi, :]
                else:
                    ps_o = psum_o_pool.tile([128, 4 * TOUT], f32, tag="ps_o",
                                            name="ps_o1")
                    for b in range(B):
                        nc.tensor.matmul(
                            ps_o[:csz, b * C1_W:(b + 1) * C1_W],
                            lhsT=e_btv[:, b, c0:c0 + csz],
                            rhs=ones_bf[:, :C1_W],
                            start=(b == 0), stop=(b == B - 1),
                            skip_group_check=True,
                        )
                        nc.tensor.matmul(
                            ps_d[:csz, :],
                            lhsT=e_btv[:, b, c0:c0 + csz],
                            rhs=ones_bf[:, 0:1],
                            start=(b == 0), stop=(b == B - 1),
                            skip_group_check=True,
                        )
                    nc.vector.reciprocal(r_cols[ci][:csz, :], ps_d[:csz, :])
                    nc.vector.tensor_scalar_mul(ats1[:], at_sb1[:], r_col1[:])
                    mb_1 = mb_pool.tile([TLOC - 128, B, C1_W], bf16, tag="mb1",
                                        name="mb1_pack")
                    nc.vector.tensor_tensor(
                        mb_1[:],
                        ps_o[:csz, 0:B * C1_W].rearrange("p (b c) -> p b c", b=B),
                        ats1[:, None, C1_T0:TOUT].to_broadcast((csz, B, C1_W)),
                        ALU.mult,
                    )
                    for b in range(B):
                        mbs[(b, 1)] = mb_1[:, b, :]

            # ---- phase 1: u.T = W.T @ x.T, tanh; s = proj.T @ tanh ----
            for ni, (n0, nsz) in enumerate(N_TILES):
                for m in range(DCH):
                    ps_u = psum_pool.tile([128, 512], f32, tag="ps_u")
                    for c in range(DCH):
                        nc.tensor.matmul(
                            ps_u[:, :nsz],
                            lhsT=w_sb[:, c, m * 128:(m + 1) * 128],
                            rhs=xt_sb[:, c, n0:n0 + nsz],
                            start=(c == 0),
                            stop=(c == DCH - 1),
                        )
                    nc.scalar.activation(tanh_sb[:, m, n0:n0 + nsz], ps_u[:, :nsz], AF.Tanh)
                ps_s_full = psum_s_pool.tile([128, 512], f32, tag="ps_s", name="ps_s")
                ps_s = ps_s_full[0:1, :]
                for m in range(DCH):
                    nc.tensor.matmul(
                        ps_s[:, :nsz],
                        lhsT=proj_sb[:, m, :],
                        rhs=tanh_sb[:, m, n0:n0 + nsz],
                        start=(m == 0),
                        stop=(m == DCH - 1),
                    )
                # exp(s) -> e_row
                nc.scalar.activation(e_row[:, n0:n0 + nsz], ps_s[:, :nsz], AF.Exp)
                if ni == len(N_TILES) - 2:
                    softmax_chunk(0)    # t 0:128 complete after second N-tile
                elif ni == len(N_TILES) - 1:
                    softmax_chunk(1)    # halo chunk after the last (small) tile

            # ---- phase 2: per-b banded matmul (chunk 0 full, halo chunk into
            # rows 96:124 only); stage PAIRS of b in one sbuf tile (adjacent
            # b's are contiguous in the output) so only 4 output DMAs are
            # needed — HWDGE descriptor preps serialize at ~625 ns each ----
            att_pair = None
            for b in range(B):
                ps_att = psum_att_pool.tile([TOUT, 512], f32, tag="ps_att")
                nc.tensor.matmul(
                    ps_att[:],
                    lhsT=mbs[(b, 0)],
                    rhs=xn_sb0[:, b * D:(b + 1) * D],
                    start=True, stop=False,
                    skip_group_check=True,
                )
                nc.tensor.matmul(
                    ps_att[C1_T0:TOUT, :],
                    lhsT=mbs[(b, 1)],
                    rhs=xn_sb1[:, b * D:(b + 1) * D],
                    start=False, stop=True,
                    skip_group_check=True,
                    tile_position=(0, C1_T0),
                )
                if b % 2 == 0:
                    att_pair = work_pool.tile([TOUT, 1024], bf16, tag="att_b",
                                              name=f"att_pair{b // 2}")
                    nc.vector.tensor_copy(att_pair[:, 0:512], ps_att[:])
                else:
                    nc.scalar.copy(att_pair[:, 512:1024], ps_att[:])
                    if (b // 2) % 2 == 0:
                        nc.sync.dma_start(
                            out_ext.ap()[:, (b - 1) * D:(b + 1) * D], att_pair[:])
                    else:
                        nc.scalar.dma_start(
                            out_ext.ap()[:, (b - 1) * D:(b + 1) * D], att_pair[:])

    nc.compile()
    return nc


def _get_nc():
    if "nc" not in _CACHE:
        _CACHE["nc"] = _build()
    return _CACHE["nc"]


def _make_in_maps(inputs, weight_W, weight_proj):
    import ml_dtypes
    bf = ml_dtypes.bfloat16

    x = np.ascontiguousarray(np.asarray(inputs, dtype=np.float32))
    w = np.asarray(weight_W, dtype=np.float32).astype(bf)
    proj = np.asarray(weight_proj, dtype=np.float32).reshape(D, 1).astype(bf)

    # constant band mask: at[i, t] = 1 if 0 <= i - t < W
    i_idx = np.arange(TLOC)[:, None]
    t_idx = np.arange(TOUT)[None, :]
    at_band = ((i_idx - t_idx >= 0) & (i_idx - t_idx < W)).astype(np.float32)

    in_maps = []
    for k in range(NCORES):
        start = k * TOUT
        avail = min(T - start, TLOC)
        shard = np.zeros((TLOC, B, D), dtype=np.float32)
        shard[:avail] = x[start:start + avail]
        shard_bf = shard.astype(bf)
        x_nat = shard_bf.reshape(TLOC, B * D)
        x_Ts = np.ascontiguousarray(shard_bf.transpose(2, 0, 1).reshape(D, NI))
        in_maps.append({
            "x_T": x_Ts,
            "x_nat": np.ascontiguousarray(x_nat),
            "w": w,
            "proj": proj,
            "at_band": at_band,
        })
    return in_maps, x


def _get_runner():
    """Persistent jitted SPMD executor for the compiled graph (one jax.jit,
    reused across kernel() calls so repeat invocations skip recompilation)."""
    if "runner" in _CACHE:
        return _CACHE["runner"]

    import jax
    from jax.sharding import Mesh, PartitionSpec
    import warnings
    with warnings.catch_warnings():
        warnings.simplefilter("ignore")
        from jax.experimental.shard_map import shard_map
    import concourse.mybir as mybir
    from concourse import bass2jax
    from concourse.bass2jax import _bass_exec_p, install_neuronx_cc_hook

    install_neuronx_cc_hook()
    nc = _get_nc()

    partition_name = nc.partition_id_tensor.name if nc.partition_id_tensor else None
    in_names, out_names, out_avals = [], [], []
    for alloc in nc.m.functions[0].allocations:
        if not isinstance(alloc, mybir.MemoryLocationSet):
            continue
        name = alloc.memorylocations[0].name
        if alloc.kind == "ExternalInput":
            if name != partition_name:
                in_names.append(name)
        elif alloc.kind == "ExternalOutput":
            out_names.append(name)
            out_avals.append(jax.core.ShapedArray(
                tuple(alloc.tensor_shape), mybir.dt.np(alloc.dtype)))
    n_params = len(in_names)
    all_names = list(in_names) + out_names
    if partition_name is not None:
        all_names.append(partition_name)

    def _body(*args):
        operands = list(args)
        if partition_name is not None:
            operands.append(bass2jax.partition_id_tensor())
        return tuple(_bass_exec_p.bind(
            *operands,
            out_avals=tuple(out_avals),
            in_names=tuple(all_names),
            out_names=tuple(out_names),
            lowering_input_output_aliases=(),
            sim_require_finite=True,
            sim_require_nnan=True,
            nc=nc,
        ))

    devices = jax.devices()[:NCORES]
    mesh = Mesh(np.asarray(devices), ("core",))
    n_outs = len(out_names)
    sharded = jax.jit(
        shard_map(_body, mesh=mesh,
                  in_specs=(PartitionSpec("core"),) * (n_params + n_outs),
                  out_specs=(PartitionSpec("core"),) * n_outs,
                  check_rep=False),
        keep_unused=True,
    )

    def run(in_maps):
        concat_in = [
            np.concatenate([np.asarray(in_maps[c][nm]) for c in range(NCORES)], axis=0)
            for nm in in_names
        ]
        concat_zeros = [
            np.zeros((NCORES * a.shape[0], *a.shape[1:]), a.dtype) for a in out_avals
        ]
        outs = sharded(*concat_in, *concat_zeros)
        jax.block_until_ready(outs)
        return [
            {nm: np.asarray(outs[i]).reshape(NCORES, *out_avals[i].shape)[c]
             for i, nm in enumerate(out_names)}
            for c in range(NCORES)
        ]

    run.body = _body
    run.mesh = mesh
    run.n_params = n_params
    run.n_outs = n_outs
    run.in_names = in_names
    run.out_avals = out_avals
    _CACHE["runner"] = run
    return run


def kernel(inputs, weight_W, weight_proj, attention_width):
    assert int(attention_width) == W
    run = _get_runner()
    in_maps, x = _make_in_maps(inputs, weight_W, weight_proj)
    results = run(in_maps)
    out = np.empty((T, B, D), dtype=np.float32)
    out[:W] = x[:W]
    for k in range(NCORES):
        out[W + k * TOUT: W + (k + 1) * TOUT] = \
            np.asarray(results[k]["out"], dtype=np.float32).reshape(TOUT, B, D)
    return out
